# revision 3
# baseline (speedup 1.0000x reference)
"""MoE balanced layer (top-2 routing, 8 experts) on 8 Trainium2 NeuronCores.

Strategy (expert parallelism, per sharding hint):
  - Host computes the (tiny) router: logits -> softmax -> top-2 -> combine
    weights + aux load-balance loss. Routing decisions are made in float64,
    which matches the reference's f32 jax routing (verified: decision margins
    are >> f32 noise for this input distribution).
  - Token dispatch (the "all-to-all") is done at input-sharding time: core e
    receives the tokens routed to expert e, already transposed to [D, C]
    (C = padded capacity), plus expert e's weights.
  - Each core runs the expert FFN for its tokens:
        hT = gelu(W1.T @ xT + b1)      (ff on partitions)
        y  = (hT.T @ W2 + b2) * w      (tokens on partitions)
    Matmuls run in float32r (TF32-style fp32 mode, full PE rate at free
    dim >= 256), accumulating in fp32 PSUM. GELU is the exact erf variant
    on the scalar engine.
  - Host combines: out[idx_e] += y_e  (same expert-order addition as the
    reference's dense combine), and returns (out, aux_loss).
"""

import math

import numpy as np

N_EXPERTS = 8
TOP_K = 2
D = 1024  # d_model
F = 4096  # d_ff
P = 128

_KERNEL_CACHE: dict = {}


def _build_device_kernel(C: int, blocks: list[tuple[int, int]]):
    import concourse.mybir as mybir
    import concourse.tile as tile
    from concourse import bacc

    F32 = mybir.dt.float32
    F32R = mybir.dt.float32r

    nc = bacc.Bacc(None, target_bir_lowering=False, debug=False)

    xT = nc.declare_dram_parameter("xT", [D, C], F32R, isOutput=False)
    w1 = nc.declare_dram_parameter("w1", [D, F], F32R, isOutput=False)
    b1c = nc.declare_dram_parameter("b1c", [P, F // P], F32, isOutput=False)
    w2 = nc.declare_dram_parameter("w2", [F, D], F32R, isOutput=False)
    b2c = nc.declare_dram_parameter("b2c", [P, D], F32, isOutput=False)
    wtc = nc.declare_dram_parameter("wtc", [P, C // P], F32, isOutput=False)
    y = nc.declare_dram_parameter("y", [C, D], F32, isOutput=True)

    xT_r = xT.rearrange("(ko p) t -> p ko t", p=P)  # [128, 8, C]
    w1_r = w1.rearrange("(ko p) f -> p ko f", p=P)  # [128, 8, 4096]
    w2_r = w2.rearrange("(ko p) d -> p ko d", p=P)  # [128, 32, 1024]
    y_r = y.rearrange("(mt p) d -> mt p d", p=P)  # [C/128, 128, 1024]

    KD = D // P  # 8  k-tiles for MM1
    KF = F // P  # 32 k-tiles for MM2
    MF = F // P  # 32 ff tiles (MM1 output)
    MG = 4  # ff tiles per weight-chunk group
    ND = D // 512  # 2  n-subtiles of 512 for MM2

    with tile.TileContext(nc) as tc:
        with (
            tc.tile_pool(name="consts", bufs=1) as consts,
            tc.tile_pool(name="xt_pool", bufs=2) as xt_pool,
            tc.tile_pool(name="w1_pool", bufs=3) as w1_pool,
            tc.tile_pool(name="w2_pool", bufs=4) as w2_pool,
            tc.tile_pool(name="ht_pool", bufs=1) as ht_pool,
            tc.tile_pool(name="yt_pool", bufs=4) as yt_pool,
            tc.tile_pool(name="ps_pool", bufs=8, space="PSUM") as ps_pool,
        ):
            b1t = consts.tile([P, F // P], F32)
            nc.sync.dma_start(out=b1t[:], in_=b1c[:])
            b2t = consts.tile([P, D], F32)
            nc.sync.dma_start(out=b2t[:], in_=b2c[:])
            wtt = consts.tile([P, C // P], F32)
            nc.sync.dma_start(out=wtt[:], in_=wtc[:])

            for t0, TB in blocks:
                # ---- load this block's tokens ----
                xt = xt_pool.tile([P, KD, TB], F32R, tag="xt")
                nc.sync.dma_start(out=xt[:], in_=xT_r[:, :, t0 : t0 + TB])

                ht = ht_pool.tile([P, MF, TB], F32R, tag="ht")

                # ---- MM1: hT[f, t] = gelu(sum_d W1[d, f] * xT[d, t] + b1[f]) ----
                for mg in range(MF // MG):
                    w1c = w1_pool.tile([P, KD, MG * P], F32R, tag="w1c")
                    nc.sync.dma_start(
                        out=w1c[:], in_=w1_r[:, :, mg * MG * P : (mg + 1) * MG * P]
                    )
                    for ml in range(MG):
                        m = mg * MG + ml
                        ps = ps_pool.tile([P, 512], F32, tag="ps")
                        for k in range(KD):
                            nc.tensor.matmul(
                                ps[:, :TB],
                                lhsT=w1c[:, k : k + 1, ml * P : (ml + 1) * P],
                                rhs=xt[:, k : k + 1, :],
                                start=(k == 0),
                                stop=(k == KD - 1),
                            )
                        nc.scalar.activation(
                            ht[:, m : m + 1, :],
                            ps[:, :TB],
                            mybir.ActivationFunctionType.Gelu,
                            bias=b1t[:, m : m + 1],
                        )

                # ---- MM2: y[t, d] = (sum_f hT[f, t] * W2[f, d] + b2[d]) * w[t] ----
                MT = TB // P
                ps2 = [
                    [
                        ps_pool.tile([P, 512], F32, tag="ps", name=f"ps2_{mt}_{n}")
                        for n in range(ND)
                    ]
                    for mt in range(MT)
                ]
                for k in range(KF):
                    w2c = w2_pool.tile([P, 1, D], F32R, tag="w2c")
                    nc.sync.dma_start(out=w2c[:], in_=w2_r[:, k : k + 1, :])
                    for mt in range(MT):
                        for n in range(ND):
                            nc.tensor.matmul(
                                ps2[mt][n][:],
                                lhsT=ht[:, k : k + 1, mt * P : (mt + 1) * P],
                                rhs=w2c[:, 0:1, n * 512 : (n + 1) * 512],
                                start=(k == 0),
                                stop=(k == KF - 1),
                            )
                for mt in range(MT):
                    mt_g = t0 // P + mt
                    for n in range(ND):
                        yt = yt_pool.tile([P, 512], F32, tag="yt")
                        nc.vector.tensor_tensor(
                            out=yt[:],
                            in0=ps2[mt][n][:],
                            in1=b2t[:, n * 512 : (n + 1) * 512],
                            op=mybir.AluOpType.add,
                        )
                        nc.vector.tensor_scalar_mul(
                            yt[:], yt[:], wtt[:, mt_g : mt_g + 1]
                        )
                        nc.sync.dma_start(
                            out=y_r[mt_g, :, n * 512 : (n + 1) * 512], in_=yt[:]
                        )

    nc.compile()
    return nc


def _make_blocks(C: int) -> list[tuple[int, int]]:
    blocks = []
    t0 = 0
    while C - t0 >= 512:
        blocks.append((t0, 512))
        t0 += 512
    if C - t0 > 0:
        blocks.append((t0, C - t0))
        t0 = C
    return blocks


def kernel(x, gate_w, W1, b1, W2, b2):
    from concourse.bass_utils import run_bass_kernel_spmd

    B, S, Dx = x.shape
    N = B * S
    x32 = np.ascontiguousarray(x, dtype=np.float32).reshape(N, D)

    # ---- router (host, float64 for robust decisions) ----
    logits = x32.astype(np.float64) @ np.asarray(gate_w, np.float64).T
    order = np.argsort(-logits, axis=-1, kind="stable")[:, :TOP_K]
    lmax = logits.max(-1, keepdims=True)
    e_l = np.exp(logits - lmax)
    probs = e_l / e_l.sum(-1, keepdims=True)
    topw = np.take_along_axis(probs, order, 1)
    rw = (topw / topw.sum(-1, keepdims=True)).astype(np.float32)

    f_i = np.bincount(order[:, 0], minlength=N_EXPERTS) / float(N)
    p_i = probs.mean(0)
    aux_loss = np.float32(0.01 * np.sum(f_i * p_i))

    # ---- build per-expert shards ----
    idxs, wts = [], []
    for e in range(N_EXPERTS):
        slot = order == e  # (N, 2); at most one True per row
        tok = slot.any(1)
        idx = np.nonzero(tok)[0]
        idxs.append(idx)
        wts.append(rw[slot].astype(np.float32))
    counts = [len(i) for i in idxs]
    C = max(256, int(math.ceil(max(counts) / 256.0)) * 256)
    blocks = _make_blocks(C)

    key = (C, tuple(blocks))
    if key not in _KERNEL_CACHE:
        _KERNEL_CACHE[key] = _build_device_kernel(C, blocks)
    nc = _KERNEL_CACHE[key]

    b1 = np.asarray(b1, np.float32)
    b2 = np.asarray(b2, np.float32)
    in_maps = []
    for e in range(N_EXPERTS):
        n_e = counts[e]
        xT_e = np.zeros((D, C), np.float32)
        xT_e[:, :n_e] = x32[idxs[e]].T
        wt_e = np.zeros(C, np.float32)
        wt_e[:n_e] = wts[e]
        wtc = np.ascontiguousarray(wt_e.reshape(C // P, P).T)
        b1c = np.ascontiguousarray(b1[e].reshape(F // P, P).T)
        b2c = np.ascontiguousarray(np.broadcast_to(b2[e], (P, D)))
        in_maps.append(
            {
                "xT": xT_e,
                "w1": np.ascontiguousarray(W1[e], dtype=np.float32),
                "b1c": b1c,
                "w2": np.ascontiguousarray(W2[e], dtype=np.float32),
                "b2c": b2c,
                "wtc": wtc,
            }
        )

    res = run_bass_kernel_spmd(nc, in_maps, core_ids=list(range(N_EXPERTS)))

    # ---- combine (host scatter-add, expert order like the reference) ----
    out = np.zeros((N, D), np.float32)
    for e in range(N_EXPERTS):
        out[idxs[e]] += res.results[e]["y"][: counts[e]]

    return out.reshape(B, S, Dx), aux_loss
